# revision 1
# baseline (speedup 1.0000x reference)
"""Causal self-attention (B=4, T=2048, C=1024, H=16) on 8 trn2 NeuronCores.

Sharding: core i = 2*b + g handles batch b (of 4) and head-group g (of 2,
8 heads each).  Inside each core:

  QKV projection runs as 3-term hi/lo fp8-e4m3 DoubleRow matmuls
  (W*x ~ Wh*xh + Wh*xl + Wl*xh, host pre-splits x and the x64-scaled
  weights; the 2^6 weight scale is folded into the exp scale / host
  gather), 0.75x the fp32r cycle cost at ~0.2% error.  QKV production is
  software-pipelined INTO the attention loop chunk by chunk so the
  Tensor engine fills the bubbles of the ACT-(exp-)bound attention
  stream instead of running a serial projection phase.

  Attention per (head, q-chunk of 512): scores computed transposed
  (S^T[k, q] = K Q^T) so the softmax axis (k) is the partition dim of
  the PV matmul; exp on ScalarE; causal handled by triangular masks on
  diagonal blocks (Pool engine) with all spans kept >= 256 so fp32r
  streams at 1 cycle/row; PV produces y^T[d, q] with row 64 = softmax
  denominator (from a ones column in V); normalization = reciprocal
  (DVE, straight from PSUM) + partition_broadcast (Pool) + one PSUM-
  direct multiply (DVE).

  The attention stream runs as one global S->exp->mask->PV pipeline: PV
  work trails the S/exp stream by LAG k-tiles ACROSS head-row and chunk
  boundaries (so the exp stream never drains at a boundary); each
  head-pair's normalization is emitted right after its last PV, and the
  previous chunk's fp32r projection is interleaved into the next chunk's
  attention as two small filler bursts per row-tile.

  Projection y^T @ W_proj rows -> per-core partial [T, C]; host sums the
  two partials per batch, divides by the 2^6 weight scale and adds
  b_proj.
"""

import os
import sys

for _p in ("/opt/trn_rl_repo", "/opt/pypackages"):
    if _p not in sys.path and os.path.isdir(_p):
        sys.path.append(_p)

import numpy as np

import concourse.bass as bass
import concourse.bacc as bacc
import concourse.mybir as mybir
from concourse.tile import TileContext
from concourse.bass_utils import run_bass_kernel_spmd

F32 = mybir.dt.float32
# fp32r streams fp32 at 1 cycle/row (vs 4 for plain fp32) when the moving
# free dim is >= 256, at ~tf32 precision.  Every producer of an fp32r matmul
# operand must itself write float32r (BIR verifier rule).
MMD = mybir.dt.float32r
FP8 = mybir.dt.float8e4
DR = mybir.MatmulPerfMode.DoubleRow

T = 2048          # tokens
C = 1024          # embed dim
D = 64            # head dim
HL = 8            # heads per core
CL = HL * D       # 512 local channels
FT = C // 128     # 8 feature tiles
NRT = CL // 128   # 4 row tiles of Q^T/K^T/y^T
NTT = T // 128    # 16 token tiles
QCH = 512         # q chunk
NCH = T // QCH    # 4 chunks
WSCALE = 64.0     # host scales W_attn (and b_attn) by 2^6 for fp8 range
SCALE = (1.0 / 8.0) / (WSCALE * WSCALE)  # 1/sqrt(D), de-scaled q*k


def build_nc():
    nc = bacc.Bacc()
    xh = nc.declare_dram_parameter("xh", [C, T], FP8, isOutput=False)
    xl = nc.declare_dram_parameter("xl", [C, T], FP8, isOutput=False)
    w8 = {}
    for nm in ("q", "k", "v"):
        for hl in ("h", "l"):
            w8[nm + hl] = nc.declare_dram_parameter(
                f"w{nm}{hl}", [C, CL], FP8, isOutput=False)
    wp = nc.declare_dram_parameter("wp", [CL, C], MMD, isOutput=False)
    bq = nc.declare_dram_parameter("bq", [CL], F32, isOutput=False)
    bk = nc.declare_dram_parameter("bk", [CL], F32, isOutput=False)
    bv = nc.declare_dram_parameter("bv", [CL], F32, isOutput=False)
    out = nc.declare_dram_parameter("out", [T, C], F32, isOutput=True)

    with TileContext(nc) as tc:
        with (
            tc.tile_pool(name="consts", bufs=1) as consts,
            tc.tile_pool(name="qkv", bufs=1) as qkvp,
            tc.tile_pool(name="qt", bufs=2) as qt_pool,
            tc.tile_pool(name="xch", bufs=2) as xch_pool,
        ):
            # ---- constants -------------------------------------------------
            # diag_mask[p, c] = 1.0 if p <= c else 0.0  (valid = k <= q)
            diag = consts.tile([128, 128], F32)
            nc.gpsimd.memset(diag, 1.0)
            nc.gpsimd.affine_select(
                out=diag, in_=diag,
                compare_op=mybir.AluOpType.is_ge,
                fill=0.0, base=0,
                pattern=[[1, 128]], channel_multiplier=-1,
            )
            zero_f32 = consts.tile([128, 128], F32, tag="zero_f32")
            nc.vector.memset(zero_f32, 0.0)
            one_f32 = consts.tile([128, 128], F32, tag="one_f32")
            nc.vector.memset(one_f32, 1.0)

# ---- input DMAs, first-needed first ----------------------------
            # HWDGE (shared by SP/ACT issues) generates descriptors serially
            # at ~630ns per DMA, so chunk 0's x and wk/wq go first; wv rides
            # the Pool SWDGE path (separate from HWDGE); biases + wp trail.
            # (x is streamed per-chunk below — both Q/K's moving operand and
            # V's stationary operand only touch the chunk's token columns.)
            w_sb = {}
            w_eng = {"q": nc.sync, "k": nc.sync, "v": nc.gpsimd}
            for nm in w8:
                w_sb[nm] = consts.tile([128, FT, CL], FP8, tag=f"w{nm}",
                                       name=f"w_{nm}")

            def load_w(nm):
                # split along f (full 512B DRAM rows per descriptor — column
                # slices would halve descriptor bandwidth)
                wr = w8[nm][:].rearrange("(f p) n -> p f n", p=128)
                for half in range(2):
                    fs = slice(half * (FT // 2), (half + 1) * (FT // 2))
                    w_eng[nm[0]].dma_start(out=w_sb[nm][:, fs, :],
                                           in_=wr[:, fs, :])

            def x_chunk(nt):
                xc = {}
                for nm, dram in (("h", xh), ("l", xl)):
                    t = xch_pool.tile([128, FT, QCH], FP8, tag=f"x{nm}",
                                      name=f"x_{nm}")
                    xr = dram[:].rearrange("(f p) n -> p f n", p=128)
                    for hv in range(2):
                        fs = slice(hv * (FT // 2), (hv + 1) * (FT // 2))
                        nc.scalar.dma_start(
                            out=t[:, fs, :],
                            in_=xr[:, fs, nt * 512:(nt + 1) * 512])
                    xc[nm] = t
                return xc

# wk+wq stream on SP back-to-back (first k/q units need them
            # complete); x0 + biases on ACT; wv on the Pool SWDGE path
            for nm in ("kh", "kl", "qh", "ql", "vh", "vl"):
                load_w(nm)
            xc0 = x_chunk(0)

# bq/bk are host-permuted to partition-major ([p, r] flat) so the
            # DMA is one contiguous 16B run per partition
            bq_sb = consts.tile([128, NRT], F32)
            bk_sb = consts.tile([128, NRT], F32)
            nc.scalar.dma_start(out=bq_sb,
                                in_=bq[:].rearrange("(p r) -> p r", p=128))
            nc.scalar.dma_start(out=bk_sb,
                                in_=bk[:].rearrange("(p r) -> p r", p=128))
            bv_sb = consts.tile([128, CL], F32)
            nc.gpsimd.dma_start(
                out=bv_sb,
                in_=bass.AP(tensor=bv, offset=0, ap=[[0, 128], [1, CL]]),
            )
            bv_r = bv_sb.rearrange("p (h d) -> p h d", d=D)
            # wp is only needed at chunk 1's projection: loaded via the
            # pump queue during chunk 0's attention
            wp_sb = consts.tile([128, NRT, C], MMD, tag="wp")

            def load_wp(rt):
                nc.scalar.dma_start(
                    out=wp_sb[:, rt, :],
                    in_=wp[:].rearrange("(r p) n -> p r n", p=128)[:, rt, :])

            # persistent K^T / V for the whole sequence
            kt_sb = qkvp.tile([128, NRT, T], MMD, tag="kt")   # K^T
            v_sb = qkvp.tile([128, NTT, HL, D + 1], MMD, tag="v")  # [V_h | 1]
            nc.vector.tensor_copy(
                v_sb[:, :, :, D:D + 1],
                one_f32.rearrange("p (t h e) -> p t h e", t=NTT, h=HL))

            with (
                tc.tile_pool(name="ps_s", bufs=2, space="PSUM") as ps_s_pool,
                tc.tile_pool(name="ps_y", bufs=2, space="PSUM") as ps_y_pool,
                tc.tile_pool(name="ps_d", bufs=2, space="PSUM") as ps_d_pool,
                tc.tile_pool(name="pt", bufs=6) as pt_pool,
                tc.tile_pool(name="yt", bufs=2) as yt_pool,
                tc.tile_pool(name="work", bufs=2) as work,
                tc.tile_pool(name="osb", bufs=2) as osb_pool,
            ):
                # ---- QKV producers (pumped into the attention stream) ------
                def mm3(ps, lhs_h, lhs_l, rhs_h, rhs_l, lslc, rslc):
                    """3-term hi/lo fp8 DoubleRow accumulation over FT."""
                    terms = ((lhs_h, rhs_h), (lhs_h, rhs_l), (lhs_l, rhs_h))
                    n = len(terms) * (FT // 2)
                    i = 0
                    for lt, rt_ in terms:
                        for f in range(0, FT, 2):
                            nc.tensor.matmul(
                                ps,
                                lhsT=lt[:, f:f + 2, lslc],
                                rhs=rt_[:, f:f + 2, rslc],
                                start=(i == 0), stop=(i == n - 1),
                                perf_mode=DR,
                            )
                            i += 1

                def q_unit(xc, qt_c, rt):
                    ps = ps_d_pool.tile([128, 512], F32, tag="d")
                    mm3(ps, w_sb["qh"], w_sb["ql"], xc["h"], xc["l"],
                        slice(rt * 128, (rt + 1) * 128), slice(0, QCH))
                    nc.vector.tensor_scalar_add(
                        qt_c[:, rt, :], ps, bq_sb[:, rt:rt + 1])

                def k_unit(xc, nt, rt):
                    ps = ps_d_pool.tile([128, 512], F32, tag="d")
                    mm3(ps, w_sb["kh"], w_sb["kl"], xc["h"], xc["l"],
                        slice(rt * 128, (rt + 1) * 128), slice(0, QCH))
                    nc.vector.tensor_scalar_add(
                        kt_sb[:, rt, nt * 512:(nt + 1) * 512],
                        ps, bk_sb[:, rt:rt + 1])

                def v_unit(xc, tt):
                    ps = ps_d_pool.tile([128, 512], F32, tag="d")
                    lt = (tt % 4) * 128
                    mm3(ps, xc["h"], xc["l"], w_sb["vh"], w_sb["vl"],
                        slice(lt, lt + 128), slice(0, CL))
                    nc.vector.tensor_add(
                        v_sb[:, tt, :, 0:D],
                        ps.rearrange("p (h d) -> p h d", d=D), bv_r)

                def qkv_chunk_units(nt, qt_c, xc=None, split_v=False):
                    if xc is None:
                        xc = x_chunk(nt)
                    units = []
                    v_units = []
                    for rt in range(NRT):
                        units.append(lambda rt=rt: k_unit(xc, nt, rt))
                        units.append(lambda rt=rt: q_unit(xc, qt_c, rt))
                        v_units.append(
                            lambda tt=4 * nt + rt: v_unit(xc, tt))
                    if split_v:
                        return units, v_units
                    for i, vu in enumerate(v_units):
                        units.insert(3 * i + 2, vu)
                    return units

                pending = []

                def pump(n=1):
                    for _ in range(n):
                        if pending:
                            pending.pop(0)()

                # ---- chunk 0 QKV: pumped per-j inside chunk 0's attention
                # (k/q/v for row j land just before S(j) needs them, so the
                # first exps start ~10us earlier)
                qt_chunks = [qt_pool.tile([128, NRT, QCH], MMD, tag="qt",
                                          name=f"qt{c}") for c in range(2)]
                pending.extend(qkv_chunk_units(0, qt_chunks[0], xc=xc0))
                pending.extend(
                    (lambda rt=rt: load_wp(rt)) for rt in range(NRT))

                def proj_half(yt_p, p_q0, ts, nb, o_sb):
                    r0 = p_q0 + ts * 128
                    ns = slice(nb * 512, (nb + 1) * 512)
                    ps_o = ps_d_pool.tile([128, 512], F32, tag="d",
                                          name="ps_o")
                    for ct in range(NRT):
                        nc.tensor.matmul(
                            ps_o,
                            lhsT=yt_p[:, ct, ts * 128:(ts + 1) * 128],
                            rhs=wp_sb[:, ct, ns],
                            start=(ct == 0), stop=(ct == NRT - 1),
                        )
                    nc.vector.tensor_copy(o_sb[:, ns], ps_o)
                    nc.sync.dma_start(out=out[r0:r0 + 128, ns],
                                      in_=o_sb[:, ns])

                def proj_group(yt_p, p_q0, ts):
                    o_sb = osb_pool.tile([128, C], F32, tag="o", name="o_sb")
                    for nb in range(2):
                        proj_half(yt_p, p_q0, ts, nb, o_sb)

                # ---- attention: one global S->exp->mask->PV pipeline -------
                # PV work trails the S/exp stream by LAG k-tiles ACROSS j and
                # chunk boundaries, so the exp stream never drains at a j
                # boundary (previously a ~3-6us ACT bubble x16).  A head-
                # pair's normalization is emitted right after its last PV.
                LAG = 4
                pipeline = []  # pending PV items

                def norm_pair(it):
                    # yt = y^T * (1/denom): recip straight from PSUM (DVE),
                    # partition_broadcast (Pool), PSUM-direct mul (DVE).
                    for hh in range(2):
                        ps_y = it["ps_ys"][hh]
                        hp = hh * D
                        rec = work.tile([1, 512], F32, tag="rec")
                        nc.vector.reciprocal(rec, ps_y[D:D + 1, :])
                        rb = work.tile([64, 512], F32, tag="rb")
                        nc.gpsimd.partition_broadcast(rb, rec)
                        nc.vector.tensor_mul(
                            it["yt_c"][hp:hp + D, it["j"], :],
                            ps_y[0:D, :], rb)

                def drain_pv():
                    it = pipeline.pop(0)
                    qs = it["qs"]
                    for hh in range(2):
                        nc.tensor.matmul(
                            it["ps_ys"][hh][0:D + 1, qs:],
                            lhsT=v_sb[:, it["kt"], 2 * it["j"] + hh, :],
                            rhs=it["pt"][:, hh, qs:],
                            start=it["start"], stop=it["stop"],
                        )
                    if it["stop"]:
                        norm_pair(it)

                prev_yt = None
                prev_q0 = 0
                for ch in range(NCH):
                    n_kt = 4 * (ch + 1)      # k-tiles 0..4ch+3 are <= chunk
                    q0 = ch * QCH
                    qt_c = qt_chunks[ch % 2]
                    if ch == NCH - 2:
                        # the last chunk's V units are only consumed at
                        # PV-lag inside it: pump them there (it has PE
                        # slack) instead of crowding this chunk
                        ku, last_v = qkv_chunk_units(
                            ch + 1, qt_chunks[(ch + 1) % 2], split_v=True)
                        pending.extend(ku)
                    elif ch < NCH - 1:
                        pending.extend(
                            qkv_chunk_units(ch + 1, qt_chunks[(ch + 1) % 2]))
                    if ch == NCH - 1:
                        pending.extend(last_v)
                    yt_c = yt_pool.tile([128, NRT, QCH], MMD, tag="yt")
                    # head pair (2j, 2j+1) = partitions 0:64 / 64:128 of
                    # row-tile j.  The two S matmuls per k-tile use disjoint
                    # PE row groups (base partition 0 vs 64).
                    for j in range(NRT):
                        if ch == 0:
                            pump(3)  # chunk 0's own k/q/v for row j
                        ps_ys = [ps_y_pool.tile([128, 512], F32, tag="y",
                                                name=f"ps_y{hh}")
                                 for hh in range(2)]
                        for kt in range(n_kt):
                            kc = slice(kt * 128, (kt + 1) * 128)
                            dj = kt - 4 * ch  # diagonal block index, if >= 0
                            # valid q-span of this k-tile within the chunk
                            # (dj==3 keeps a 256-wide span for fp32r rate;
                            # the extra cols are zeroed by diag2).
                            qs = min(dj, 2) * 128 if dj > 0 else 0
                            ps_s = ps_s_pool.tile([128, 2, 512], F32, tag="s")
                            for hh in range(2):
                                hp = hh * D
                                nc.tensor.matmul(
                                    ps_s[:, hh, qs:],
                                    lhsT=kt_sb[hp:hp + D, j, kc],
                                    rhs=qt_c[hp:hp + D, j, qs:],
                                    start=True, stop=True,
                                )
                            eqs = dj * 128 if dj > 0 else 0
                            pt = pt_pool.tile([128, 2, 512], MMD, tag="pt")
                            nc.scalar.activation(
                                pt[:, :, eqs:], ps_s[:, :, eqs:],
                                mybir.ActivationFunctionType.Exp,
                                scale=SCALE)
                            if dj == 3:
                                # exp skips the fully-masked [256:384]; zero
                                # it in pt via copies (Memset can't write
                                # f32r; this copy pattern is device-proven)
                                nc.gpsimd.tensor_copy(pt[:, 0, 256:384],
                                                      zero_f32)
                                nc.gpsimd.tensor_copy(pt[:, 1, 256:384],
                                                      zero_f32)
                            if dj >= 0:
                                # triangular mask on the diagonal 128-block
                                # (Pool engine); dj==3 also zeroes [256:384].
                                for hh in range(2):
                                    blk = pt[:, hh,
                                             dj * 128:(dj + 1) * 128]
                                    nc.gpsimd.tensor_mul(blk, blk, diag)
                            pipeline.append(dict(
                                kt=kt, j=j, qs=qs, pt=pt, ps_ys=ps_ys,
                                yt_c=yt_c, start=(kt == 0),
                                stop=(kt == n_kt - 1)))
                            if len(pipeline) > LAG:
                                drain_pv()
                            if kt == LAG and prev_yt is not None:
                                # previous chunk's projection; at kt==LAG the
                                # pipeline drain has already emitted the
                                # previous chunk's last norm (emitting it at
                                # kt<LAG would deadlock the PE queue on it)
                                o_sb = osb_pool.tile([128, C], F32, tag="o",
                                                     name="o_sb")
                                proj_half(prev_yt, prev_q0, j, 0, o_sb)
                            if kt == LAG + 2 and prev_yt is not None:
                                proj_half(prev_yt, prev_q0, j, 1, o_sb)
                            pump(1)
                    prev_yt, prev_q0 = yt_c, q0
                while pipeline:
                    drain_pv()
                while pending:
                    pump(1)
                # tail: last chunk's projection
                for ts in range(QCH // 128):
                    proj_group(prev_yt, prev_q0, ts)
    nc.compile()
    return nc


_NC = None


def _get_nc():
    global _NC
    if _NC is None:
        _NC = build_nc()
    return _NC


def _split8(a):
    import ml_dtypes
    hi = np.ascontiguousarray(a).astype(ml_dtypes.float8_e4m3)
    lo = (a - hi.astype(np.float32)).astype(ml_dtypes.float8_e4m3)
    return hi, lo


def _make_in_maps(x, W_attn, b_attn, W_proj):
    x = np.ascontiguousarray(np.asarray(x, dtype=np.float32))
    W_attn = np.asarray(W_attn, dtype=np.float32) * WSCALE
    b_attn = np.asarray(b_attn, dtype=np.float32) * WSCALE
    W_proj = np.asarray(W_proj, dtype=np.float32)

    xs = [_split8(x[b].T) for b in range(4)]
    wsplit = {}
    for g in range(2):
        s = slice(g * CL, (g + 1) * CL)
        for i, nm in enumerate(("q", "k", "v")):
            wh, wl = _split8(W_attn[:, i * C:(i + 1) * C][:, s])
            wsplit[(g, nm)] = (wh, wl)

    in_maps = []
    for core in range(8):
        b, g = core // 2, core % 2
        s = slice(g * CL, (g + 1) * CL)
        m = {
            "xh": xs[b][0],
            "xl": xs[b][1],
            "wp": np.ascontiguousarray(W_proj[s, :]),
            # bq/bk permuted to partition-major (see kernel DMA comment)
            "bq": np.ascontiguousarray(
                b_attn[0 * C:1 * C][s].reshape(NRT, 128).T.ravel()),
            "bk": np.ascontiguousarray(
                b_attn[1 * C:2 * C][s].reshape(NRT, 128).T.ravel()),
            "bv": np.ascontiguousarray(b_attn[2 * C:3 * C][s]),
        }
        for nm in ("q", "k", "v"):
            m[f"w{nm}h"], m[f"w{nm}l"] = wsplit[(g, nm)]
        in_maps.append(m)
    return in_maps


def _gather(results, b_proj):
    b_proj = np.asarray(b_proj, dtype=np.float32)
    out = np.empty((4, T, C), dtype=np.float32)
    inv = np.float32(1.0 / WSCALE)
    for b in range(4):
        out[b] = (results[2 * b]["out"] + results[2 * b + 1]["out"]) * inv \
            + b_proj
    return out


def run(x, W_attn, b_attn, W_proj, b_proj, trace=False):
    """Reference path via run_bass_kernel_spmd (re-traces every call)."""
    nc = _get_nc()
    in_maps = _make_in_maps(x, W_attn, b_attn, W_proj)
    res = run_bass_kernel_spmd(nc, in_maps, list(range(8)), trace=trace)
    return _gather(res.results, b_proj), res


class _Runner:
    """Cached PJRT executor: builds the sharded jit once, reuses it.

    No output donation: the kernel writes every element of "out", so the
    pre-zeroed output operand run_bass_kernel_spmd donates is unnecessary.
    """

    def __init__(self, nc, n_cores=8):
        import jax
        from jax.experimental.shard_map import shard_map
        from jax.sharding import Mesh, NamedSharding, PartitionSpec
        from concourse.bass2jax import (
            _bass_exec_p, install_neuronx_cc_hook, partition_id_tensor)

        install_neuronx_cc_hook()
        self.jax = jax
        self.nc = nc
        self.n_cores = n_cores
        in_names, out_names, out_avals = [], [], []
        for alloc in nc.m.functions[0].allocations:
            if not isinstance(alloc, mybir.MemoryLocationSet):
                continue
            name = alloc.memorylocations[0].name
            if alloc.kind == "ExternalInput":
                if name != "partition_id":
                    in_names.append(name)
            elif alloc.kind == "ExternalOutput":
                out_names.append(name)
                out_avals.append(jax.core.ShapedArray(
                    tuple(alloc.tensor_shape), mybir.dt.np(alloc.dtype)))
        self.in_names = in_names
        self.out_names = out_names
        self.out_avals = out_avals
        all_in = in_names + out_names + ["partition_id"]
        n_ops = len(in_names) + len(out_names)

        def _body(*args):
            outs = _bass_exec_p.bind(
                *args, partition_id_tensor(),
                out_avals=tuple(out_avals),
                in_names=tuple(all_in),
                out_names=tuple(out_names),
                lowering_input_output_aliases=(),
                sim_require_finite=True,
                sim_require_nnan=True,
                nc=nc,
            )
            return tuple(outs)

        devices = jax.devices()[:n_cores]
        self.mesh = Mesh(np.asarray(devices), ("core",))
        spec = PartitionSpec("core")
        self.sharding = NamedSharding(self.mesh, spec)
        self.fn = jax.jit(
            shard_map(_body, mesh=self.mesh, in_specs=(spec,) * n_ops,
                      out_specs=(spec,) * len(out_names), check_rep=False),
            keep_unused=True)
        # device-resident zeros, reused every call (read-only operand)
        self.zero_out = [
            jax.device_put(
                np.zeros((n_cores * av.shape[0], *av.shape[1:]), av.dtype),
                self.sharding)
            for av in out_avals
        ]

    def __call__(self, in_maps):
        n = self.n_cores
        concat_in = [
            np.concatenate([np.asarray(in_maps[c][name]) for c in range(n)],
                           axis=0)
            for name in self.in_names
        ]
        outs = self.fn(*concat_in, *self.zero_out)
        out = np.asarray(outs[0]).reshape(n, *self.out_avals[0].shape)
        return [{self.out_names[0]: out[c]} for c in range(n)]


_RUNNER = None


def _get_runner():
    global _RUNNER
    if _RUNNER is None:
        _RUNNER = _Runner(_get_nc())
    return _RUNNER


def kernel(x, W_attn, b_attn, W_proj, b_proj):
    in_maps = _make_in_maps(x, W_attn, b_attn, W_proj)
    try:
        results = _get_runner()(in_maps)
    except Exception:
        res = run_bass_kernel_spmd(_get_nc(), in_maps, list(range(8)))
        results = res.results
    return _gather(results, b_proj)



# revision 27
# speedup vs baseline: 1.1005x; 1.1005x over previous
"""Causal self-attention (B=4, T=2048, C=1024, H=16) on 8 trn2 NeuronCores.

Sharding: core i = 2*b + g handles batch b (of 4) and head-group g (of 2,
8 heads each).  Inside each core:

  QKV projection runs as hi/lo fp8-e4m3 DoubleRow matmuls: q/k use 2 terms
  (Wh*xh + Wh*xl -- W quantization error ~2.5% rms is dominated by the fp8
  storage of q/k themselves), v uses 3 terms (its error hits the output
  directly).  QKV production is software-pipelined INTO the attention loop
  chunk by chunk.

  Attention per (head, q-chunk of 512): scores computed transposed
  (S^T[k, q] = K Q^T) as single-fp8 DoubleRow matmuls with the d=64
  contraction folded to [32 partitions x 2 row-slots] (W_attn's q/k blocks
  are host-permuted so the QKV PSUM comes out in that order; the DVE bias
  add writes the folded fp8 layout directly).  exp on ScalarE writes pt
  straight to fp8; causal masking is multiplicative on the fp8 diagonal
  blocks (Pool) plus fp8 memsets for pair-gap regions.  PV runs as fp8
  DoubleRow matmuls over k-tile PAIRS (contraction 256) with V stored as
  fp8 hi+lo (2 accumulation terms); row 64 of V (hi) is a ones column so
  the PV output carries the softmax denominator.  Normalization =
  reciprocal (DVE, straight from PSUM, both heads at once) +
  partition_broadcast (Pool) + one PSUM-direct multiply per head (DVE).

  The attention stream runs as one global S->exp->mask->PV pipeline: PV
  pair-work trails the S/exp stream by LAG pairs ACROSS head-row and chunk
  boundaries, each head-pair's normalization is emitted right after its
  last PV, and the previous chunk's fp32r projection is interleaved into
  the next chunk's attention as two small filler bursts per row-tile.

  Projection y^T @ W_proj rows -> per-core partial [T, C]; host sums the
  two partials per batch, divides by the 2^6 weight scale and adds b_proj.
"""

import os
import sys

for _p in ("/opt/trn_rl_repo", "/opt/pypackages"):
    if _p not in sys.path and os.path.isdir(_p):
        sys.path.append(_p)

import numpy as np

import concourse.bass as bass
import concourse.bacc as bacc
import concourse.mybir as mybir
from concourse.tile import TileContext
from concourse.bass_utils import run_bass_kernel_spmd

F32 = mybir.dt.float32
# fp32r streams fp32 at 1 cycle/row (vs 4 for plain fp32) when the moving
# free dim is >= 256, at ~tf32 precision.  Every producer of an fp32r matmul
# operand must itself write float32r (BIR verifier rule).
MMD = mybir.dt.float32r
FP8 = mybir.dt.float8e4
DR = mybir.MatmulPerfMode.DoubleRow

T = 2048          # tokens
C = 1024          # embed dim
D = 64            # head dim
HL = 8            # heads per core
CL = HL * D       # 512 local channels
FT = C // 128     # 8 feature tiles
NRT = CL // 128   # 4 row tiles of Q^T/K^T/y^T
NTT = T // 128    # 16 token tiles
VP = 80           # padded V row (65 used; DoubleRow needs 16B-mult steps)
QCH = 512         # q chunk
NCH = T // QCH    # 4 chunks
WSCALE = 64.0     # host scales W_attn (and b_attn) by 2^6 for fp8 range
SCALE = (1.0 / 8.0) / (WSCALE * WSCALE)  # 1/sqrt(D), de-scaled q*k


DEBUG = False


def build_nc():
    nc = bacc.Bacc()
    xh = nc.declare_dram_parameter("xh", [C, T], FP8, isOutput=False)
    xl = nc.declare_dram_parameter("xl", [C, T], FP8, isOutput=False)
    w8 = {}
    for nm in ("qh", "kh", "vh", "vl"):
        w8[nm] = nc.declare_dram_parameter(
            f"w{nm}", [C, CL], FP8, isOutput=False)
    wp = nc.declare_dram_parameter("wp", [CL, C], MMD, isOutput=False)
    bq = nc.declare_dram_parameter("bq", [CL], F32, isOutput=False)
    bk = nc.declare_dram_parameter("bk", [CL], F32, isOutput=False)
    bv = nc.declare_dram_parameter("bv", [CL], F32, isOutput=False)
    out = nc.declare_dram_parameter("out", [T, C], F32, isOutput=True)
    if DEBUG:
        dbg_kt = nc.declare_dram_parameter(
            "dbg_kt", [128, NRT, 2, T], FP8, isOutput=True)
        dbg_qt = nc.declare_dram_parameter(
            "dbg_qt", [128, NRT, 2, QCH], FP8, isOutput=True)
        dbg_v = nc.declare_dram_parameter(
            "dbg_v", [128, NTT, 2, HL, VP], FP8, isOutput=True)
        dbg_pt = nc.declare_dram_parameter(
            "dbg_pt", [128, 2, 2, 512], FP8, isOutput=True)
        dbg_y = nc.declare_dram_parameter(
            "dbg_y", [128, 2, 512], F32, isOutput=True)

    with TileContext(nc) as tc:
        with (
            tc.tile_pool(name="consts", bufs=1) as consts,
            tc.tile_pool(name="qkv", bufs=1) as qkvp,
            tc.tile_pool(name="qt", bufs=2) as qt_pool,
            tc.tile_pool(name="xch", bufs=2) as xch_pool,
        ):
            # ---- constants -------------------------------------------------
            # diag8[p, c] = 1.0 if p <= c else 0.0  (valid = k <= q), fp8
            diag8 = consts.tile([128, 128], FP8)
            nc.gpsimd.memset(diag8, 1.0)
            nc.gpsimd.affine_select(
                out=diag8, in_=diag8,
                compare_op=mybir.AluOpType.is_ge,
                fill=0.0, base=0,
                pattern=[[1, 128]], channel_multiplier=-1,
            )

            # ---- input DMAs, first-needed first ----------------------------
            # HWDGE (shared by SP/ACT issues) generates descriptors serially
            # at ~630ns per DMA, so chunk 0's x and wk/wq go first; wv rides
            # the Pool SWDGE path (separate from HWDGE); biases + wp trail.
            w_sb = {}
            w_eng = {"q": nc.sync, "k": nc.sync, "v": nc.sync}
            for nm in w8:
                w_sb[nm] = consts.tile([128, FT, CL], FP8, tag=f"w{nm}",
                                       name=f"w_{nm}")

            def load_w(nm):
                # split along f (full 512B DRAM rows per descriptor — column
                # slices would halve descriptor bandwidth)
                wr = w8[nm][:].rearrange("(f p) n -> p f n", p=128)
                for half in range(2):
                    fs = slice(half * (FT // 2), (half + 1) * (FT // 2))
                    w_eng[nm[0]].dma_start(out=w_sb[nm][:, fs, :],
                                           in_=wr[:, fs, :])

            def x_chunk(nt):
                # chunk 0 rides the ACT->HWDGE queue (free at startup);
                # later chunks issue on SP so their issue cost never blocks
                # the ACT sequencer between exps (Pool SWDGE costs ~1.2us
                # of Pool-engine time per DMA; SP is idle).
                eng = nc.scalar if nt == 0 else nc.sync
                xc = {}
                for nm, dram in (("h", xh), ("l", xl)):
                    t = xch_pool.tile([128, FT, QCH], FP8, tag=f"x{nm}",
                                      name=f"x_{nm}")
                    xr = dram[:].rearrange("(f p) n -> p f n", p=128)
                    for hv in range(2):
                        fs = slice(hv * (FT // 2), (hv + 1) * (FT // 2))
                        eng.dma_start(
                            out=t[:, fs, :],
                            in_=xr[:, fs, nt * 512:(nt + 1) * 512])
                    xc[nm] = t
                return xc

            # DMA-engine service is serial (~0.73us per 256KB): order by
            # first need: wk, x0, wq, biases, then wv (first v unit is the
            # 3rd pumped unit).
            load_w("kh")
            xc0 = x_chunk(0)
            load_w("qh")

            # bq/bk are host-permuted to partition-major ([p, r] flat, p =
            # the folded PSUM channel order) so the DMA is one contiguous
            # run per partition
            bq_sb = consts.tile([128, NRT], F32)
            bk_sb = consts.tile([128, NRT], F32)
            nc.gpsimd.dma_start(out=bq_sb,
                                in_=bq[:].rearrange("(p r) -> p r", p=128))
            nc.gpsimd.dma_start(out=bk_sb,
                                in_=bk[:].rearrange("(p r) -> p r", p=128))
            bv_sb = consts.tile([128, CL], F32)
            nc.gpsimd.dma_start(
                out=bv_sb,
                in_=bass.AP(tensor=bv, offset=0, ap=[[0, 128], [1, CL]]),
            )
            for nm in ("vh", "vl"):
                load_w(nm)
            # wp is only needed at chunk 1's projection: loaded via the
            # pump queue during chunk 0's attention (SP issue: the ACT
            # sequencer must stay free for the exp stream)
            wp_sb = consts.tile([128, NRT, C], MMD, tag="wp")

            def load_wp(rt):
                nc.sync.dma_start(
                    out=wp_sb[:, rt, :],
                    in_=wp[:].rearrange("(r p) n -> p r n", p=128)[:, rt, :])

            # persistent K^T / V for the whole sequence.
            # kt8/qt8 fold d=64 as [32 partitions x 2 free slots] for the
            # fp8 DoubleRow S matmul: head 2j+hh lives at partition base
            # 32*hh (only partitions 0:64 carry data; base 96 is not a
            # legal matmul operand base), index [j, slot, t].
            kt8 = qkvp.tile([128, NRT, 2, T], FP8, tag="kt")
            # v8: [tok-part, tt, hi/lo, head, D+1 (pad VP)]; col 64 of hi
            # is the ones column (denominator), zero in lo.
            v8 = qkvp.tile([128, NTT, 2, HL, VP], FP8, tag="v")
            nc.gpsimd.memset(v8[:, :, 0, :, D:D + 1], 1.0)
            nc.gpsimd.memset(v8[:, :, 1, :, D:D + 1], 0.0)

            with (
                tc.tile_pool(name="ps_s", bufs=2, space="PSUM") as ps_s_pool,
                tc.tile_pool(name="ps_y", bufs=1, space="PSUM") as ps_y_pool,
                tc.tile_pool(name="ps_d", bufs=2, space="PSUM") as ps_d_pool,
                tc.tile_pool(name="pt", bufs=16) as pt_pool,
                tc.tile_pool(name="yt", bufs=2) as yt_pool,
                tc.tile_pool(name="vt", bufs=2) as vt_pool,
                tc.tile_pool(name="work", bufs=2) as work,
                tc.tile_pool(name="osb", bufs=4) as osb_pool,
            ):
                # ---- QKV producers (pumped into the attention stream) ------
                def mm_hl(ps, terms, lslc, rslc):
                    """hi/lo fp8 DoubleRow accumulation over FT."""
                    n = len(terms) * (FT // 2)
                    i = 0
                    for lt, rt_ in terms:
                        for f in range(0, FT, 2):
                            nc.tensor.matmul(
                                ps,
                                lhsT=lt[:, f:f + 2, lslc],
                                rhs=rt_[:, f:f + 2, rslc],
                                start=(i == 0), stop=(i == n - 1),
                                perf_mode=DR,
                            )
                            i += 1

                def qk_unit(xc, dst, b_sb, rt, cols):
                    # 2-term: W quantization error only; the PSUM channel
                    # order is host-permuted to the folded [slot, head, d32]
                    # layout, so two DVE ops write fp8 slots directly.
                    ps = ps_d_pool.tile([128, 512], F32, tag="d")
                    wname = "qh" if dst is not kt8 else "kh"
                    mm_hl(ps, ((w_sb[wname], xc["h"]), (w_sb[wname], xc["l"])),
                          slice(rt * 128, (rt + 1) * 128), slice(0, QCH))
                    for s in range(2):
                        nc.vector.tensor_scalar_add(
                            dst[0:64, rt, s, cols],
                            ps[64 * s:64 * s + 64, :],
                            b_sb[64 * s:64 * s + 64, rt:rt + 1])

                def v_unit(xc, tt):
                    ps = ps_d_pool.tile([128, 512], F32, tag="d")
                    lt = (tt % 4) * 128
                    mm_hl(ps, ((xc["h"], w_sb["vh"]), (xc["h"], w_sb["vl"]),
                               (xc["l"], w_sb["vh"])),
                          slice(lt, lt + 128), slice(0, CL))
                    vt = vt_pool.tile([128, CL], F32, tag="vt")
                    nc.vector.tensor_add(vt, ps, bv_sb)
                    vr = vt.rearrange("p (h d) -> p h d", d=D)
                    nc.vector.tensor_copy(v8[:, tt, 0, :, 0:D], vr)
                    nc.gpsimd.tensor_tensor(
                        v8[:, tt, 1, :, 0:D], vr, v8[:, tt, 0, :, 0:D],
                        mybir.AluOpType.subtract)

                def qkv_chunk_units(nt, qt_c, xc=None, split_v=False):
                    if xc is None:
                        xc = x_chunk(nt)
                    k_units, q_units, v_units = [], [], []
                    cols = slice(nt * 512, (nt + 1) * 512)
                    for rt in range(NRT):
                        k_units.append(lambda rt=rt: qk_unit(
                            xc, kt8, bk_sb, rt, cols))
                        q_units.append(lambda rt=rt: qk_unit(
                            xc, qt_c, bq_sb, rt, slice(0, QCH)))
                        v_units.append(
                            lambda tt=4 * nt + rt: v_unit(xc, tt))
                    if nt > 0:
                        # prefetch: q first (the next chunk's S stalls only
                        # on q; K is persistent and new k/v tiles are only
                        # read late in the next chunk's rows)
                        units = list(q_units)
                        for i in range(NRT):
                            units.append(k_units[i])
                            if not split_v:
                                units.append(v_units[i])
                        if split_v:
                            return units, v_units
                        return units
                    units = []
                    for i in range(NRT):
                        units.append(k_units[i])
                        units.append(q_units[i])
                        units.append(v_units[i])
                    if split_v:
                        return units, v_units
                    # chunk 0 consumes v row by row: keep k,q,v interleaved
                    return units

                pending = []

                def pump(n=1):
                    for _ in range(n):
                        if pending:
                            pending.pop(0)()

                # ---- chunk 0 QKV: pumped per-j inside chunk 0's attention
                qt_chunks = [qt_pool.tile([128, NRT, 2, QCH], FP8, tag="qt",
                                          name=f"qt{c}") for c in range(2)]
                pending.extend(qkv_chunk_units(0, qt_chunks[0], xc=xc0))
                pending.extend(
                    (lambda rt=rt: load_wp(rt)) for rt in range(NRT))

                def proj_half(yt_p, p_q0, ts, nb, o_sb):
                    r0 = p_q0 + ts * 128
                    ns = slice(nb * 512, (nb + 1) * 512)
                    ps_o = ps_d_pool.tile([128, 512], F32, tag="d",
                                          name="ps_o")
                    for ct in range(NRT):
                        nc.tensor.matmul(
                            ps_o,
                            lhsT=yt_p[:, ct, ts * 128:(ts + 1) * 128],
                            rhs=wp_sb[:, ct, ns],
                            start=(ct == 0), stop=(ct == NRT - 1),
                        )
                    nc.vector.tensor_copy(o_sb[:, ns], ps_o)
                    nc.sync.dma_start(out=out[r0:r0 + 128, ns],
                                      in_=o_sb[:, ns])

                def proj_group(yt_p, p_q0, ts):
                    o_sb = osb_pool.tile([128, C], F32, tag="o", name="o_sb")
                    for nb in range(2):
                        proj_half(yt_p, p_q0, ts, nb, o_sb)

                # ---- attention: one global S->exp->mask->PV pipeline -------
                # PV pair-work trails the S/exp stream by LAG pairs ACROSS j
                # and chunk boundaries.  A head-pair's normalization is
                # emitted right after its last PV.
                LAG = 4  # in k-tile PAIRS (= 4 k-tiles)
                pipeline = []  # pending PV pair items

                def norm_pair(it):
                    # yt = y^T * (1/denom): one recip for both heads straight
                    # from PSUM (DVE), partition_broadcast (Pool), PSUM-
                    # direct multiply per head (DVE).  (A PE-matmul
                    # broadcast into PSUM is illegal downstream: the mul
                    # may read only one non-scalar PSUM input.)
                    ps_y = it["ps_y"]
                    rec = work.tile([1, 2, 512], F32, tag="rec")
                    nc.vector.reciprocal(rec, ps_y[D:D + 1, :, :])
                    rb = work.tile([64, 2, 512], F32, tag="rb")
                    nc.gpsimd.partition_broadcast(rb, rec)
                    for hh in range(2):
                        nc.vector.tensor_mul(
                            it["yt_c"][hh * D:hh * D + D, it["j"], :],
                            ps_y[0:D, hh, :], rb[:, hh, :])

                def drain_pv():
                    it = pipeline.pop(0)
                    qs = it["qs"]
                    m = it["m"]
                    h0 = 2 * it["j"]
                    for hh in range(2):
                        for hl in range(2):
                            nc.tensor.matmul(
                                it["ps_y"][0:D + 1, hh, qs:],
                                lhsT=v8[:, 2 * m:2 * m + 2, hl,
                                        h0 + hh, 0:D + 1],
                                rhs=it["pt"][:, :, hh, qs:],
                                start=it["start"] and hl == 0,
                                stop=it["stop"] and hl == 1,
                                perf_mode=DR,
                            )
                    if it["stop"]:
                        norm_pair(it)

                prev_yt = None
                prev_q0 = 0
                for ch in range(NCH):
                    n_kt = 4 * (ch + 1)      # k-tiles 0..4ch+3 are <= chunk
                    q0 = ch * QCH
                    qt_c = qt_chunks[ch % 2]
                    if ch == NCH - 2:
                        # the last chunk's V units are only consumed at
                        # PV-lag inside it: pump them there (it has PE
                        # slack) instead of crowding this chunk
                        ku, last_v = qkv_chunk_units(
                            ch + 1, qt_chunks[(ch + 1) % 2], split_v=True)
                        pending.extend(ku)
                    elif ch < NCH - 1:
                        pending.extend(
                            qkv_chunk_units(ch + 1, qt_chunks[(ch + 1) % 2]))
                    if ch == NCH - 1:
                        pending.extend(last_v)
                    if ch > 0:
                        # drain the previous chunk's trailing pairs NOW:
                        # their norms must land before this chunk's kt==4/6
                        # projection anchors read the previous chunk's yt.
                        while pipeline:
                            drain_pv()
                    yt_c = yt_pool.tile([128, NRT, QCH], MMD, tag="yt")
                    # head pair (2j, 2j+1): folded S layout at partition
                    # bases 64*(j%2) + {0, 32}; PV pairs accumulate into one
                    # [128, 2, 512] PSUM tile (hh along free dim).
                    for j in range(NRT):
                        if ch == 0:
                            pump(3)  # chunk 0's own k/q/v for row j
                        ps_y = ps_y_pool.tile([128, 2, 512], F32, tag="y")
                        pt_pair = None
                        for kt in range(n_kt):
                            kc = slice(kt * 128, (kt + 1) * 128)
                            e = kt % 2
                            dj = kt - 4 * ch  # diagonal block index, if >= 0
                            # valid q-span of this k-tile within the chunk
                            qs = min(dj, 2) * 128 if dj > 0 else 0
                            ps_s = ps_s_pool.tile([128, 2, 512], F32, tag="s")
                            for hh in range(2):
                                p0 = 32 * hh
                                nc.tensor.matmul(
                                    ps_s[:, hh, qs:],
                                    lhsT=kt8[p0:p0 + 32, j, :, kc],
                                    rhs=qt_c[p0:p0 + 32, j, :, qs:],
                                    start=True, stop=True,
                                    perf_mode=DR,
                                )
                            if e == 0:
                                pt_pair = pt_pool.tile([128, 2, 2, 512], FP8,
                                                       tag="pt")
                                if DEBUG and ch == 0 and j == 0 and kt == 0:
                                    dbg_pt_src = pt_pair
                            eqs = dj * 128 if dj > 0 else 0
                            nc.scalar.activation(
                                pt_pair[:, e, :, eqs:], ps_s[:, :, eqs:],
                                mybir.ActivationFunctionType.Exp,
                                scale=SCALE)
                            if dj == 1:
                                # pair-gap [0:128] of the odd tile (PV pair
                                # reads from the even tile's q-start)
                                nc.gpsimd.memset(pt_pair[:, e, :, 0:128], 0.0)
                            if dj == 3:
                                # exp skips the fully-masked [256:384]
                                nc.gpsimd.memset(
                                    pt_pair[:, e, :, 256:384], 0.0)
                            if dj >= 0:
                                # triangular mask on the diagonal 128-block
                                for hh in range(2):
                                    blk = pt_pair[:, e, hh,
                                                  dj * 128:(dj + 1) * 128]
                                    nc.gpsimd.tensor_mul(blk, blk, diag8)
                            if e == 1:
                                m = kt // 2
                                dj_e = 2 * m - 4 * ch
                                qsp = 256 if dj_e == 2 else 0
                                pipeline.append(dict(
                                    m=m, j=j, qs=qsp, pt=pt_pair, ps_y=ps_y,
                                    yt_c=yt_c, start=(m == 0),
                                    stop=(m == n_kt // 2 - 1)))
                                lag = (0 if ch == NCH - 1 and j == NRT - 1
                                       else LAG)
                                while len(pipeline) > lag:
                                    drain_pv()
                            if kt == 4 and prev_yt is not None:
                                # previous chunk's projection; by kt==4 the
                                # pipeline drain has already emitted the
                                # previous chunk's last norm (emitting it
                                # earlier would deadlock the PE queue)
                                o_sb = osb_pool.tile([128, C], F32, tag="o",
                                                     name="o_sb")
                                proj_half(prev_yt, prev_q0, j, 0, o_sb)
                            if kt == 6 and prev_yt is not None:
                                proj_half(prev_yt, prev_q0, j, 1, o_sb)
                            pump(1)
                        if DEBUG and ch == 0 and j == 0:
                            while pipeline:
                                drain_pv()
                            nc.sync.dma_start(out=dbg_pt[:],
                                              in_=dbg_pt_src)
                            dbg_y_sb = work.tile([128, 2, 512], F32,
                                                 tag="dbgy")
                            nc.vector.tensor_copy(dbg_y_sb, ps_y)
                            nc.sync.dma_start(out=dbg_y[:], in_=dbg_y_sb)
                        if ch > 0:
                            # flush this row's PV + norm now: trailing them
                            # across the row boundary delays the norm, whose
                            # ps_y/pt WARs then stall the ACT exp stream.
                            # (chunk 0 must trail: its v units are pumped
                            # row by row and PE queues in-order.)
                            while pipeline:
                                drain_pv()
                    prev_yt, prev_q0 = yt_c, q0
                while pipeline:
                    drain_pv()
                while pending:
                    pump(1)
                # tail: last chunk's projection
                for ts in range(QCH // 128):
                    proj_group(prev_yt, prev_q0, ts)
                if DEBUG:
                    nc.sync.dma_start(out=dbg_kt[:], in_=kt8)
                    nc.sync.dma_start(out=dbg_qt[:], in_=qt_chunks[0])
                    nc.sync.dma_start(out=dbg_v[:], in_=v8)
    nc.compile()
    return nc


_NC = None


def _get_nc():
    global _NC
    if _NC is None:
        _NC = build_nc()
    return _NC


def _split8(a):
    import ml_dtypes
    hi = np.ascontiguousarray(a).astype(ml_dtypes.float8_e4m3)
    lo = (a - hi.astype(np.float32)).astype(ml_dtypes.float8_e4m3)
    return hi, lo


def _fold_perm(w):
    """Permute the 512 q/k out-channels of one head-group block so each
    128-column unit comes out of the QKV matmul in the folded
    [slot(d-half), head(2), d%32] order the fp8 DoubleRow S layout needs."""
    C_ = w.shape[0]
    return np.ascontiguousarray(
        w.reshape(C_, NRT, 2, 2, 32).transpose(0, 1, 3, 2, 4)
        .reshape(C_, CL))


def _fold_perm_b(b):
    return b.reshape(NRT, 2, 2, 32).transpose(0, 2, 1, 3).reshape(CL)


def _make_in_maps(x, W_attn, b_attn, W_proj):
    x = np.ascontiguousarray(np.asarray(x, dtype=np.float32))
    W_attn = np.asarray(W_attn, dtype=np.float32) * WSCALE
    b_attn = np.asarray(b_attn, dtype=np.float32) * WSCALE
    W_proj = np.asarray(W_proj, dtype=np.float32)

    xs = [_split8(x[b].T) for b in range(4)]
    wsplit = {}
    for g in range(2):
        s = slice(g * CL, (g + 1) * CL)
        for i, nm in enumerate(("q", "k", "v")):
            blk = W_attn[:, i * C:(i + 1) * C][:, s]
            if nm in ("q", "k"):
                wh, _ = _split8(_fold_perm(blk))
                wsplit[(g, nm)] = (wh, None)
            else:
                wsplit[(g, nm)] = _split8(blk)

    in_maps = []
    for core in range(8):
        b, g = core // 2, core % 2
        s = slice(g * CL, (g + 1) * CL)
        # bq/bk: folded channel order, then partition-major (see kernel)
        bqv = _fold_perm_b(b_attn[0 * C:1 * C][s])
        bkv = _fold_perm_b(b_attn[1 * C:2 * C][s])
        m = {
            "xh": xs[b][0],
            "xl": xs[b][1],
            "wqh": wsplit[(g, "q")][0],
            "wkh": wsplit[(g, "k")][0],
            "wvh": wsplit[(g, "v")][0],
            "wvl": wsplit[(g, "v")][1],
            "wp": np.ascontiguousarray(W_proj[s, :]),
            "bq": np.ascontiguousarray(
                bqv.reshape(NRT, 128).T.ravel()),
            "bk": np.ascontiguousarray(
                bkv.reshape(NRT, 128).T.ravel()),
            "bv": np.ascontiguousarray(b_attn[2 * C:3 * C][s]),
        }
        in_maps.append(m)
    return in_maps


def _gather(results, b_proj):
    b_proj = np.asarray(b_proj, dtype=np.float32)
    out = np.empty((4, T, C), dtype=np.float32)
    inv = np.float32(1.0 / WSCALE)
    for b in range(4):
        out[b] = (results[2 * b]["out"] + results[2 * b + 1]["out"]) * inv \
            + b_proj
    return out


def run(x, W_attn, b_attn, W_proj, b_proj, trace=False):
    """Reference path via run_bass_kernel_spmd (re-traces every call)."""
    nc = _get_nc()
    in_maps = _make_in_maps(x, W_attn, b_attn, W_proj)
    res = run_bass_kernel_spmd(nc, in_maps, list(range(8)), trace=trace)
    return _gather(res.results, b_proj), res


class _Runner:
    """Cached PJRT executor: builds the sharded jit once, reuses it.

    No output donation: the kernel writes every element of "out", so the
    pre-zeroed output operand run_bass_kernel_spmd donates is unnecessary.
    """

    def __init__(self, nc, n_cores=8):
        import jax
        from jax.experimental.shard_map import shard_map
        from jax.sharding import Mesh, NamedSharding, PartitionSpec
        from concourse.bass2jax import (
            _bass_exec_p, install_neuronx_cc_hook, partition_id_tensor)

        install_neuronx_cc_hook()
        self.jax = jax
        self.nc = nc
        self.n_cores = n_cores
        in_names, out_names, out_avals = [], [], []
        for alloc in nc.m.functions[0].allocations:
            if not isinstance(alloc, mybir.MemoryLocationSet):
                continue
            name = alloc.memorylocations[0].name
            if alloc.kind == "ExternalInput":
                if name != "partition_id":
                    in_names.append(name)
            elif alloc.kind == "ExternalOutput":
                out_names.append(name)
                out_avals.append(jax.core.ShapedArray(
                    tuple(alloc.tensor_shape), mybir.dt.np(alloc.dtype)))
        self.in_names = in_names
        self.out_names = out_names
        self.out_avals = out_avals
        all_in = in_names + out_names + ["partition_id"]
        n_ops = len(in_names) + len(out_names)

        def _body(*args):
            outs = _bass_exec_p.bind(
                *args, partition_id_tensor(),
                out_avals=tuple(out_avals),
                in_names=tuple(all_in),
                out_names=tuple(out_names),
                lowering_input_output_aliases=(),
                sim_require_finite=True,
                sim_require_nnan=True,
                nc=nc,
            )
            return tuple(outs)

        devices = jax.devices()[:n_cores]
        self.mesh = Mesh(np.asarray(devices), ("core",))
        spec = PartitionSpec("core")
        self.sharding = NamedSharding(self.mesh, spec)
        self.fn = jax.jit(
            shard_map(_body, mesh=self.mesh, in_specs=(spec,) * n_ops,
                      out_specs=(spec,) * len(out_names), check_rep=False),
            keep_unused=True)
        # device-resident zeros, reused every call (read-only operand)
        self.zero_out = [
            jax.device_put(
                np.zeros((n_cores * av.shape[0], *av.shape[1:]), av.dtype),
                self.sharding)
            for av in out_avals
        ]

    def __call__(self, in_maps):
        n = self.n_cores
        concat_in = [
            np.concatenate([np.asarray(in_maps[c][name]) for c in range(n)],
                           axis=0)
            for name in self.in_names
        ]
        outs = self.fn(*concat_in, *self.zero_out)
        out = np.asarray(outs[0]).reshape(n, *self.out_avals[0].shape)
        return [{self.out_names[0]: out[c]} for c in range(n)]


_RUNNER = None


def _get_runner():
    global _RUNNER
    if _RUNNER is None:
        _RUNNER = _Runner(_get_nc())
    return _RUNNER


def kernel(x, W_attn, b_attn, W_proj, b_proj):
    in_maps = _make_in_maps(x, W_attn, b_attn, W_proj)
    try:
        results = _get_runner()(in_maps)
    except Exception:
        res = run_bass_kernel_spmd(_get_nc(), in_maps, list(range(8)))
        results = res.results
    return _gather(results, b_proj)
